# revision 33
# baseline (speedup 1.0000x reference)
"""Trainium2 Bass kernel for CrossAttention (B=8, N=M=2048, C=1024), fp32 in/out.

Sharding: data-parallel — one batch element per NeuronCore (8 cores).

Key optimizations over the straightforward version:
  * Mask packing: the reference applies a binary mask over support positions
    AFTER softmax, so masked positions only matter for the softmax denominator.
    The host permutes support rows so unmasked rows come first (1012 -> padded
    to 1024 = mt_u tiles); v is built and p@v contracted over that half only,
    while the s = q@k^T logits (and the exp-sum denominator) still span all M.
    This halves the v-build, the p transposes, and the p@v contraction.
  * bf16 operand storage everywhere (fp32 PSUM accumulation): same PE rate as
    f32r on TRN2 (1 cyc/row) but half the DMA bytes and SBUF footprint, which
    lets qT, kT, v, and the full o tensor stay SBUF-resident.  o in its
    natural [n, d] layout is exactly the lhsT the output projection needs
    (the swapaxes/reshape fold makes proj contract over o's row index), so
    no DRAM bounce is needed.
  * Q phase first: its first matmul needs only a small wq slice + one x
    chunk, so the tensor engine starts ~3us earlier, and the entire V/K
    working set (support^T, wv, wk) prefetches during Q's compute.
  * Few large DMA instructions (multi-dim APs) — each dma_start costs
    ~650ns of serial issue on the sync sequencer.
  * The attention inner loop is software-pipelined: transposes + p@v of
    group g-1 are emitted after the s matmuls of group g, hiding the exp
    latency from the tensor engine.

Per-core computation (batch b):
  qT[d, n] = (x[b] @ wq^T)^T
  v[m, d]  = (support_perm[b] @ wv^T) * mask_perm[m]   (m < mv only)
  kT[d, m] = (support_perm[b] @ wk^T)^T                (all m)
  p = exp(SCALE * q k^T)  (no max subtraction: logits ~ N(0, 8), safe fp32)
  o[n, d] = (p[:, :mv] @ v) / rowsum_all_m(p)
  out[2t+i, d'] = sum_c o[1024 i + c, t] * proj_w[d', c] + proj_b[d']
"""

import sys

sys.path.insert(0, "/opt/trn_rl_repo")

import numpy as np
import ml_dtypes

import concourse.bass as bass
import concourse.tile as tile
from concourse import bacc, mybir
from concourse.bass_utils import run_bass_kernel_spmd
from concourse.masks import make_identity

F32 = mybir.dt.float32
BF16 = mybir.dt.bfloat16
AF = mybir.ActivationFunctionType
NPBF = ml_dtypes.bfloat16

B, N, M, C = 8, 2048, 2048, 1024
CT = C // 128          # 8 c-tiles (contraction / channel partition tiles)
MT = M // 128          # 16 m-tiles
SCALE = (C // 8) ** -0.5
NCHUNK = 256           # q rows computed per chunk in the q phase
MS = 512               # m-chunk for kv build and the s matmul free dim

_CACHE = {}


def _build_program(mt_u):
    nc = bacc.Bacc(
        "TRN2",
        target_bir_lowering=False,
        debug=False,
        enable_asserts=False,
        num_devices=8,
    )

    xT = nc.dram_tensor("xT", [128, N // NCHUNK, CT, NCHUNK], BF16, kind="ExternalInput")
    sT = nc.dram_tensor("sT", [128, M // MS, CT, MS], BF16, kind="ExternalInput")
    wqT = nc.dram_tensor("wqT", [128, CT * C], BF16, kind="ExternalInput")
    wkT = nc.dram_tensor("wkT", [128, CT * C], BF16, kind="ExternalInput")
    wvT = nc.dram_tensor("wvT", [128, CT * C], BF16, kind="ExternalInput")
    pwT = nc.dram_tensor("pwT", [128, CT * C], BF16, kind="ExternalInput")
    maskf = nc.dram_tensor("maskf", [128, mt_u], F32, kind="ExternalInput")
    biasb = nc.dram_tensor("biasb", [128, C], F32, kind="ExternalInput")
    out = nc.dram_tensor("out", [N, C], F32, kind="ExternalOutput")

    with tile.TileContext(nc, pool_alloc_mode="queue") as tc:
        _trace_kernel(tc, mt_u, xT, sT, wqT, wkT, wvT, pwT, maskf, biasb, out)
    nc.compile()
    return nc


def _trace_kernel(tc, mt_u, xT, sT, wqT, wkT, wvT, pwT, maskf, biasb, out):
    nc = tc.nc
    mv = mt_u * 128

    from contextlib import ExitStack

    with ExitStack() as ctx:
        persist = ctx.enter_context(tc.tile_pool(name="persist", bufs=1))

        ident0 = persist.tile([128, 128], F32, tag="ident0")
        make_identity(nc, ident0[:])
        ident = persist.tile([128, 128], BF16, tag="ident")
        nc.scalar.copy(ident[:], ident0[:])

        # long-lived tiles (LIFO pool stack: these close only at the end)
        qtp = ctx.enter_context(tc.tile_pool(name="qtp", bufs=1))
        qt = qtp.tile([128, CT, N], BF16, tag="qt")
        vp = ctx.enter_context(tc.tile_pool(name="vp", bufs=1))
        v = vp.tile([128, mt_u, C], BF16, tag="v")
        kTp = ctx.enter_context(tc.tile_pool(name="kTp", bufs=1))
        kT = kTp.tile([128, CT, M], BF16, tag="kT")
        o_p = ctx.enter_context(tc.tile_pool(name="op", bufs=1))
        o_sb = o_p.tile([128, (N // 128) * C], BF16, tag="o_sb")
        pwp = ctx.enter_context(tc.tile_pool(name="pwp", bufs=1, side="right"))
        pw = pwp.tile([128, CT * C], BF16, tag="pw")

        # support^T loaded once, shared by the V and K phases; window-major
        # so each window's DMA is one contiguous 8KB/partition transfer
        st_ctx = ctx.enter_context(ExitStack())
        stp = st_ctx.enter_context(tc.tile_pool(name="stp", bufs=1))
        st = stp.tile([128, M // MS, CT, MS], BF16, tag="st")

        maskt = persist.tile([128, mt_u], F32, tag="maskt")
        bias = persist.tile([128, C], F32, tag="bias")

        # ---------------- phase Q: qT[d, n] = (x @ wq^T)^T -----------------
        # wq is dt-major: [p, dt*1024 + ct*128 + dd]
        with (
            tc.tile_pool(name="wqp", bufs=1, side="right") as wqp,
            tc.tile_pool(name="xq", bufs=6) as xqp,
            tc.tile_pool(name="qps", bufs=3, space="PSUM") as qps,
        ):
            wq = wqp.tile([128, CT * C], BF16, tag="wq")

            # ramp-up: x chunk 0 + wq in dt-sized pieces (paced with the Q
            # loop's consumption), then the remaining x chunks and the K/V
            # working set.  Every DMA in this block is dependency-free, so
            # nothing head-of-line-blocks the serial DMA issue queue.
            xqs = []
            xq0 = xqp.tile([128, CT, NCHUNK], BF16, tag="xq")
            nc.sync.dma_start(xq0[:, :, :], xT[:, 0, :, :])
            xqs.append(xq0)
            for dt in range(CT):
                nc.sync.dma_start(
                    wq[:, dt * 1024:(dt + 1) * 1024],
                    wqT[:, dt * 1024:(dt + 1) * 1024],
                )
            for i in range(1, 6):
                t = xqp.tile([128, CT, NCHUNK], BF16, tag="xq")
                nc.sync.dma_start(t[:, :, :], xT[:, i, :, :])
                xqs.append(t)
            nc.sync.dma_start(maskt[:], maskf[:])
            nc.sync.dma_start(bias[:], biasb[:])
            for mc in range(M // MS):
                nc.sync.dma_start(st[:, mc, :, :], sT[:, mc, :, :])

            for nch in range(N // NCHUNK):
                if nch < 6:
                    xq = xqs[nch]
                else:
                    xq = xqp.tile([128, CT, NCHUNK], BF16, tag="xq")
                    nc.sync.dma_start(xq[:, :, :], xT[:, nch, :, :])
                for dt in range(CT):
                    ps = qps.tile([128, NCHUNK], F32, tag="qps")
                    for ct in range(CT):
                        nc.tensor.matmul(
                            ps[:],
                            lhsT=wq[:, dt * C + ct * 128: dt * C + (ct + 1) * 128],
                            rhs=xq[:, ct, :],
                            start=(ct == 0),
                            stop=(ct == CT - 1),
                        )
                    nc.scalar.copy(
                        qt[:, dt, nch * NCHUNK:(nch + 1) * NCHUNK], ps[:]
                    )

        # ---------------- phases K then V (shared support^T) ---------------
        with (
            tc.tile_pool(name="wkp", bufs=1, side="right") as wkp,
            tc.tile_pool(name="wvp", bufs=1, side="right") as wvp,
            tc.tile_pool(name="kvps", bufs=3, space="PSUM") as kvps,
        ):
            wk = wkp.tile([128, CT * C], BF16, tag="wk")
            wv = wvp.tile([128, CT * C], BF16, tag="wv")
            # wk first (K consumes dt-blocks in order), wv + pw behind it
            nc.sync.dma_start(wk[:, 0:2048], wkT[:, 0:2048])
            nc.sync.dma_start(wk[:, 2048:4096], wkT[:, 2048:4096])
            nc.sync.dma_start(wk[:, 4096:8192], wkT[:, 4096:8192])
            nc.sync.dma_start(wv[:, 0:4096], wvT[:, 0:4096])
            nc.sync.dma_start(wv[:, 4096:8192], wvT[:, 4096:8192])
            nc.sync.dma_start(pw[:, 0:4096], pwT[:, 0:4096])
            nc.sync.dma_start(pw[:, 4096:8192], pwT[:, 4096:8192])

            # K: kT[d, m] = (support @ wk^T)^T; wk is dt-major like wq
            for mc in range(M // MS):
                for dt in range(CT):
                    ps = kvps.tile([128, MS], F32, tag="kvps")
                    for ct in range(CT):
                        nc.tensor.matmul(
                            ps[:],
                            lhsT=wk[:, dt * C + ct * 128: dt * C + (ct + 1) * 128],
                            rhs=st[:, mc, ct, :],
                            start=(ct == 0),
                            stop=(ct == CT - 1),
                        )
                    nc.scalar.copy(
                        kT[:, dt, mc * MS:(mc + 1) * MS], ps[:]
                    )

            # V: v[m, d] = mask * (support @ wv^T); wv is dc-major; first mv
            # rows only
            for mt in range(mt_u):
                mc, j = divmod(mt, MS // 128)
                for dc in range(C // 512):
                    ps = kvps.tile([128, 512], F32, tag="kvps")
                    for ct in range(CT):
                        nc.tensor.matmul(
                            ps[:],
                            lhsT=st[:, mc, ct, j * 128:(j + 1) * 128],
                            rhs=wv[:, dc * 4096 + ct * 512: dc * 4096 + (ct + 1) * 512],
                            start=(ct == 0),
                            stop=(ct == CT - 1),
                        )
                    nc.vector.tensor_scalar_mul(
                        v[:, mt, dc * 512:(dc + 1) * 512],
                        ps[:],
                        maskt[:, mt:mt + 1],
                    )

        st_ctx.close()

        # ---------------- attention: s / exp / transpose / p@v -------------
        with (
            tc.tile_pool(name="sps", bufs=2, space="PSUM") as sps,
            tc.tile_pool(name="ptps", bufs=2, space="PSUM") as ptps,
            tc.tile_pool(name="ops", bufs=1, space="PSUM") as ops,
            tc.tile_pool(name="psb", bufs=3) as psbp,
            tc.tile_pool(name="ptsb", bufs=2) as ptsbp,
            tc.tile_pool(name="stat", bufs=4) as statp,
        ):
            def transpose_and_pv(g, p_sb, o_ps):
                pt_ps = ptps.tile([128, MS], BF16, tag="ptps")
                for j in range(MS // 128):
                    nc.tensor.transpose(
                        pt_ps[:, j * 128:(j + 1) * 128],
                        p_sb[:, j * 128:(j + 1) * 128],
                        ident[:],
                    )
                pt_sb = ptsbp.tile([128, MS], BF16, tag="ptsb")
                nc.vector.tensor_copy(pt_sb[:], pt_ps[:])
                for j in range(MS // 128):
                    mt = g * (MS // 128) + j
                    for dc in range(C // 512):
                        nc.tensor.matmul(
                            o_ps[:, dc * 512:(dc + 1) * 512],
                            lhsT=pt_sb[:, j * 128:(j + 1) * 128],
                            rhs=v[:, mt, dc * 512:(dc + 1) * 512],
                            start=(mt == 0),
                            stop=(mt == mt_u - 1),
                        )

            for ntile in range(N // 128):
                partials = statp.tile([128, 4], F32, tag="partials")
                o_ps = ops.tile([128, C], F32, tag="ops")
                for g in range(M // MS):
                    s_ps = sps.tile([128, MS], F32, tag="sps")
                    for dt in range(CT):
                        nc.tensor.matmul(
                            s_ps[:],
                            lhsT=qt[:, dt, ntile * 128:(ntile + 1) * 128],
                            rhs=kT[:, dt, g * MS:(g + 1) * MS],
                            start=(dt == 0),
                            stop=(dt == CT - 1),
                        )
                    p_sb = psbp.tile([128, MS], BF16, tag="psb")
                    nc.scalar.activation(
                        p_sb[:], s_ps[:], AF.Exp,
                        scale=float(SCALE),
                        accum_out=partials[:, g:g + 1],
                    )
                    if g * MS < mv:
                        transpose_and_pv(g, p_sb, o_ps)
                denom = statp.tile([128, 1], F32, tag="denom")
                nc.vector.reduce_sum(
                    denom[:], partials[:], axis=mybir.AxisListType.X
                )
                recip = statp.tile([128, 1], F32, tag="recip")
                nc.vector.reciprocal(recip[:], denom[:])
                nc.vector.tensor_scalar_mul(
                    o_sb[:, ntile * C:(ntile + 1) * C], o_ps[:], recip[:]
                )
        # ---------------- projection with the swapaxes/reshape fold --------
        # out[2t+i, d'] = sum_c o[1024 i + c, t] pw[d', c] + bias: the lhsT
        # blocks are o's SBUF row-tiles as produced by attention.  pw is
        # dc-major like wv.  (Interleaving proj into the attention loop
        # creates tiny PE gaps that each reset the pstate ramp — net loss.)
        with (
            tc.tile_pool(name="fps", bufs=4, space="PSUM") as fps,
            tc.tile_pool(name="fsb", bufs=6) as fsbp,
        ):
            out_v = out[:].rearrange("(t two) d -> two t d", two=2)
            for i in range(2):
                for dc in range(C // 512):
                    for tt in range(CT):
                        ps = fps.tile([128, 512], F32, tag="fps")
                        for ct in range(CT):
                            nc.tensor.matmul(
                                ps[:],
                                lhsT=o_sb[:, (CT * i + ct) * C + tt * 128: (CT * i + ct) * C + (tt + 1) * 128],
                                rhs=pw[:, dc * 4096 + ct * 512: dc * 4096 + (ct + 1) * 512],
                                start=(ct == 0),
                                stop=(ct == CT - 1),
                            )
                        f_sb = fsbp.tile([128, 512], F32, tag="fsb")
                        nc.vector.tensor_add(
                            f_sb[:], ps[:], bias[:, dc * 512:(dc + 1) * 512]
                        )
                        nc.sync.dma_start(
                            out_v[i, tt * 128:(tt + 1) * 128, dc * 512:(dc + 1) * 512],
                            f_sb[:],
                        )


def _prep_w_lhs(w):
    # lhsT weights (wk, wq): dt-major [p, dt*1024 + ct*128 + dd]
    wt = w.T.reshape(CT, 128, CT, 128)          # [ct, p, dt, dd]
    return np.ascontiguousarray(
        wt.transpose(1, 2, 0, 3).reshape(128, CT * C).astype(NPBF)
    )


def _prep_w_rhs(w):
    # rhs weights (wv, pw): dc-major [p, dc*4096 + ct*512 + dd]
    wt = w.T.reshape(CT, 128, C // 512, 512)    # [ct, p, dc, dd]
    return np.ascontiguousarray(
        wt.transpose(1, 2, 0, 3).reshape(128, CT * C).astype(NPBF)
    )


def _prep_act(a, win):
    # a [rows, C] -> a.T [C, rows] grouped window-major as [p, rows/win, ct,
    # win] so each window's DMA is one contiguous per-partition run
    n = a.shape[0]
    at = a.T.reshape(CT, 128, n // win, win)    # [ct, p, w, win]
    return np.ascontiguousarray(
        at.transpose(1, 2, 0, 3).astype(NPBF)
    )


def _mask_perm(attn_mask):
    # permutation packing unmasked support rows first; tile count for packed v
    mask = np.asarray(attn_mask)
    perm = np.argsort(mask == 0, kind="stable")
    cnt = int((mask != 0).sum())
    mt_u = max(1, min(MT, -(-cnt // 128)))
    return perm, mt_u


def prep_in_maps(x, support, attn_mask, qkv_w, proj_w, proj_b):
    x = np.asarray(x, dtype=np.float32)
    support = np.asarray(support, dtype=np.float32)
    attn_mask = np.asarray(attn_mask)
    qkv_w = np.asarray(qkv_w, dtype=np.float32)
    proj_w = np.asarray(proj_w, dtype=np.float32)
    proj_b = np.asarray(proj_b, dtype=np.float32)

    perm, mt_u = _mask_perm(attn_mask)
    maskp = attn_mask[perm].astype(np.float32)

    wq = _prep_w_lhs(qkv_w[:C])
    wk = _prep_w_lhs(qkv_w[C:2 * C])
    wv = _prep_w_rhs(qkv_w[2 * C:])
    pw = _prep_w_rhs(proj_w)
    maskf = np.ascontiguousarray(
        maskp[:mt_u * 128].reshape(mt_u, 128).T
    )
    biasb = np.ascontiguousarray(np.broadcast_to(proj_b, (128, C)))

    in_maps = []
    for b in range(B):
        in_maps.append({
            "xT": _prep_act(x[b], NCHUNK),
            "sT": _prep_act(support[b][perm], MS),
            "wqT": wq,
            "wkT": wk,
            "wvT": wv,
            "pwT": pw,
            "maskf": maskf,
            "biasb": biasb,
        })
    return in_maps


def kernel(x, support, attn_mask, qkv_w, proj_w, proj_b):
    _, mt_u = _mask_perm(attn_mask)
    if ("nc", mt_u) not in _CACHE:
        _CACHE[("nc", mt_u)] = _build_program(mt_u)
        _CACHE["nc"] = _CACHE[("nc", mt_u)]
    nc = _CACHE[("nc", mt_u)]

    in_maps = prep_in_maps(x, support, attn_mask, qkv_w, proj_w, proj_b)
    res = run_bass_kernel_spmd(nc, in_maps, core_ids=list(range(B)))
    return np.stack([res.results[b]["out"] for b in range(B)], axis=0)


# revision 34
# speedup vs baseline: 1.2742x; 1.2742x over previous
"""Trainium2 Bass kernel for CrossAttention (B=8, N=M=2048, C=1024), fp32 in/out.

Sharding: data-parallel — one batch element per NeuronCore (8 cores).

Key optimizations over the straightforward version:
  * Mask packing: the reference applies a binary mask over support positions
    AFTER softmax, so masked positions only matter for the softmax denominator.
    The host permutes support rows so unmasked rows come first (1012 -> padded
    to 1024 = mt_u tiles); v is built and p@v contracted over that half only,
    while the s = q@k^T logits (and the exp-sum denominator) still span all M.
    This halves the v-build, the p transposes, and the p@v contraction.
  * bf16 operand storage everywhere (fp32 PSUM accumulation): same PE rate as
    f32r on TRN2 (1 cyc/row) but half the DMA bytes and SBUF footprint, which
    lets qT, kT, v, and the full o tensor stay SBUF-resident.  o in its
    natural [n, d] layout is exactly the lhsT the output projection needs
    (the swapaxes/reshape fold makes proj contract over o's row index), so
    no DRAM bounce is needed.
  * Q phase first: its first matmul needs only a small wq slice + one x
    chunk, so the tensor engine starts ~3us earlier, and the entire V/K
    working set (support^T, wv, wk) prefetches during Q's compute.
  * Few large DMA instructions (multi-dim APs) — each dma_start costs
    ~650ns of serial issue on the sync sequencer.
  * The attention inner loop is software-pipelined: transposes + p@v of
    group g-1 are emitted after the s matmuls of group g, hiding the exp
    latency from the tensor engine.

Per-core computation (batch b):
  qT[d, n] = (x[b] @ wq^T)^T
  v[m, d]  = (support_perm[b] @ wv^T) * mask_perm[m]   (m < mv only)
  kT[d, m] = (support_perm[b] @ wk^T)^T                (all m)
  p = exp(SCALE * q k^T)  (no max subtraction: logits ~ N(0, 8), safe fp32)
  o[n, d] = (p[:, :mv] @ v) / rowsum_all_m(p)
  out[2t+i, d'] = sum_c o[1024 i + c, t] * proj_w[d', c] + proj_b[d']
"""

import sys

sys.path.insert(0, "/opt/trn_rl_repo")

import numpy as np
import ml_dtypes

import concourse.bass as bass
import concourse.tile as tile
from concourse import bacc, mybir
from concourse.bass_utils import run_bass_kernel_spmd
from concourse.masks import make_identity

F32 = mybir.dt.float32
BF16 = mybir.dt.bfloat16
AF = mybir.ActivationFunctionType
NPBF = ml_dtypes.bfloat16

B, N, M, C = 8, 2048, 2048, 1024
CT = C // 128          # 8 c-tiles (contraction / channel partition tiles)
MT = M // 128          # 16 m-tiles
SCALE = (C // 8) ** -0.5
NCHUNK = 256           # q rows computed per chunk in the q phase
MS = 512               # m-chunk for kv build and the s matmul free dim

_CACHE = {}


def _build_program(mt_u):
    nc = bacc.Bacc(
        "TRN2",
        target_bir_lowering=False,
        debug=False,
        enable_asserts=False,
        num_devices=8,
    )

    xT = nc.dram_tensor("xT", [128, N // NCHUNK, CT, NCHUNK], BF16, kind="ExternalInput")
    sT = nc.dram_tensor("sT", [128, M // MS, CT, MS], BF16, kind="ExternalInput")
    wqT = nc.dram_tensor("wqT", [128, CT * C], BF16, kind="ExternalInput")
    wkT = nc.dram_tensor("wkT", [128, CT * C], BF16, kind="ExternalInput")
    wvT = nc.dram_tensor("wvT", [128, CT * C], BF16, kind="ExternalInput")
    pwT = nc.dram_tensor("pwT", [128, CT * C], BF16, kind="ExternalInput")
    maskf = nc.dram_tensor("maskf", [128, mt_u], F32, kind="ExternalInput")
    biasb = nc.dram_tensor("biasb", [128, C], F32, kind="ExternalInput")
    out = nc.dram_tensor("out", [N, C], F32, kind="ExternalOutput")

    with tile.TileContext(nc, pool_alloc_mode="queue") as tc:
        _trace_kernel(tc, mt_u, xT, sT, wqT, wkT, wvT, pwT, maskf, biasb, out)
    nc.compile()
    return nc


def _trace_kernel(tc, mt_u, xT, sT, wqT, wkT, wvT, pwT, maskf, biasb, out):
    nc = tc.nc
    mv = mt_u * 128

    from contextlib import ExitStack

    with ExitStack() as ctx:
        persist = ctx.enter_context(tc.tile_pool(name="persist", bufs=1))

        ident0 = persist.tile([128, 128], F32, tag="ident0")
        make_identity(nc, ident0[:])
        ident = persist.tile([128, 128], BF16, tag="ident")
        nc.scalar.copy(ident[:], ident0[:])

        # long-lived tiles (LIFO pool stack: these close only at the end)
        qtp = ctx.enter_context(tc.tile_pool(name="qtp", bufs=1))
        qt = qtp.tile([128, CT, N], BF16, tag="qt")
        vp = ctx.enter_context(tc.tile_pool(name="vp", bufs=1))
        v = vp.tile([128, mt_u, C], BF16, tag="v")
        kTp = ctx.enter_context(tc.tile_pool(name="kTp", bufs=1))
        kT = kTp.tile([128, CT, M], BF16, tag="kT")
        o_p = ctx.enter_context(tc.tile_pool(name="op", bufs=1))
        o_sb = o_p.tile([128, (N // 128) * C], BF16, tag="o_sb")
        pwp = ctx.enter_context(tc.tile_pool(name="pwp", bufs=1, side="right"))
        pw = pwp.tile([128, CT * C], BF16, tag="pw")

        # support^T loaded once, shared by the V and K phases; window-major
        # so each window's DMA is one contiguous 8KB/partition transfer
        st_ctx = ctx.enter_context(ExitStack())
        stp = st_ctx.enter_context(tc.tile_pool(name="stp", bufs=1))
        st = stp.tile([128, M // MS, CT, MS], BF16, tag="st")

        maskt = persist.tile([128, mt_u], F32, tag="maskt")
        bias = persist.tile([128, C], F32, tag="bias")

        # ---------------- phase Q: qT[d, n] = (x @ wq^T)^T -----------------
        # wq is dt-major: [p, dt*1024 + ct*128 + dd]
        with (
            tc.tile_pool(name="wqp", bufs=1, side="right") as wqp,
            tc.tile_pool(name="xq", bufs=6) as xqp,
            tc.tile_pool(name="qps", bufs=3, space="PSUM") as qps,
        ):
            wq = wqp.tile([128, CT * C], BF16, tag="wq")

            # ramp-up: x chunk 0 + wq in dt-sized pieces (paced with the Q
            # loop's consumption), then the remaining x chunks and the K/V
            # working set.  Every DMA in this block is dependency-free, so
            # nothing head-of-line-blocks the serial DMA issue queue.
            xqs = []
            xq0 = xqp.tile([128, CT, NCHUNK], BF16, tag="xq")
            nc.sync.dma_start(xq0[:, :, :], xT[:, 0, :, :])
            xqs.append(xq0)
            for dt in range(CT):
                nc.sync.dma_start(
                    wq[:, dt * 1024:(dt + 1) * 1024],
                    wqT[:, dt * 1024:(dt + 1) * 1024],
                )
            for i in range(1, 6):
                t = xqp.tile([128, CT, NCHUNK], BF16, tag="xq")
                nc.sync.dma_start(t[:, :, :], xT[:, i, :, :])
                xqs.append(t)
            nc.sync.dma_start(maskt[:], maskf[:])
            nc.sync.dma_start(bias[:], biasb[:])
            for mc in range(M // MS):
                nc.sync.dma_start(st[:, mc, :, :], sT[:, mc, :, :])

            for nch in range(N // NCHUNK):
                if nch < 6:
                    xq = xqs[nch]
                else:
                    xq = xqp.tile([128, CT, NCHUNK], BF16, tag="xq")
                    nc.sync.dma_start(xq[:, :, :], xT[:, nch, :, :])
                for dt in range(CT):
                    ps = qps.tile([128, NCHUNK], F32, tag="qps")
                    for ct in range(CT):
                        nc.tensor.matmul(
                            ps[:],
                            lhsT=wq[:, dt * C + ct * 128: dt * C + (ct + 1) * 128],
                            rhs=xq[:, ct, :],
                            start=(ct == 0),
                            stop=(ct == CT - 1),
                        )
                    nc.scalar.copy(
                        qt[:, dt, nch * NCHUNK:(nch + 1) * NCHUNK], ps[:]
                    )

        # ---------------- phases K then V (shared support^T) ---------------
        with (
            tc.tile_pool(name="wkp", bufs=1, side="right") as wkp,
            tc.tile_pool(name="wvp", bufs=1, side="right") as wvp,
            tc.tile_pool(name="kvps", bufs=3, space="PSUM") as kvps,
        ):
            wk = wkp.tile([128, CT * C], BF16, tag="wk")
            wv = wvp.tile([128, CT * C], BF16, tag="wv")
            # wk first (K consumes dt-blocks in order), wv + pw behind it
            nc.sync.dma_start(wk[:, 0:2048], wkT[:, 0:2048])
            nc.sync.dma_start(wk[:, 2048:4096], wkT[:, 2048:4096])
            nc.sync.dma_start(wk[:, 4096:8192], wkT[:, 4096:8192])
            nc.sync.dma_start(wv[:, 0:4096], wvT[:, 0:4096])
            nc.sync.dma_start(wv[:, 4096:8192], wvT[:, 4096:8192])
            nc.sync.dma_start(pw[:, 0:4096], pwT[:, 0:4096])
            nc.sync.dma_start(pw[:, 4096:8192], pwT[:, 4096:8192])

            # K: kT[d, m] = (support @ wk^T)^T; wk is dt-major like wq
            for mc in range(M // MS):
                for dt in range(CT):
                    ps = kvps.tile([128, MS], F32, tag="kvps")
                    for ct in range(CT):
                        nc.tensor.matmul(
                            ps[:],
                            lhsT=wk[:, dt * C + ct * 128: dt * C + (ct + 1) * 128],
                            rhs=st[:, mc, ct, :],
                            start=(ct == 0),
                            stop=(ct == CT - 1),
                        )
                    nc.scalar.copy(
                        kT[:, dt, mc * MS:(mc + 1) * MS], ps[:]
                    )

            # V: v[m, d] = mask * (support @ wv^T); wv is dc-major; first mv
            # rows only
            for mt in range(mt_u):
                mc, j = divmod(mt, MS // 128)
                for dc in range(C // 512):
                    ps = kvps.tile([128, 512], F32, tag="kvps")
                    for ct in range(CT):
                        nc.tensor.matmul(
                            ps[:],
                            lhsT=st[:, mc, ct, j * 128:(j + 1) * 128],
                            rhs=wv[:, dc * 4096 + ct * 512: dc * 4096 + (ct + 1) * 512],
                            start=(ct == 0),
                            stop=(ct == CT - 1),
                        )
                    nc.vector.tensor_scalar_mul(
                        v[:, mt, dc * 512:(dc + 1) * 512],
                        ps[:],
                        maskt[:, mt:mt + 1],
                    )

        st_ctx.close()

        # ---------------- attention: s / exp / transpose / p@v -------------
        with (
            tc.tile_pool(name="sps", bufs=2, space="PSUM") as sps,
            tc.tile_pool(name="ptps", bufs=2, space="PSUM") as ptps,
            tc.tile_pool(name="ops", bufs=2, space="PSUM") as ops,
            tc.tile_pool(name="psb", bufs=3) as psbp,
            tc.tile_pool(name="ptsb", bufs=2) as ptsbp,
            tc.tile_pool(name="stat", bufs=4) as statp,
        ):
            def transpose_and_pv(g, p_sb, o_ps):
                pt_ps = ptps.tile([128, MS], BF16, tag="ptps")
                for j in range(MS // 128):
                    nc.tensor.transpose(
                        pt_ps[:, j * 128:(j + 1) * 128],
                        p_sb[:, j * 128:(j + 1) * 128],
                        ident[:],
                    )
                pt_sb = ptsbp.tile([128, MS], BF16, tag="ptsb")
                nc.vector.tensor_copy(pt_sb[:], pt_ps[:])
                for j in range(MS // 128):
                    mt = g * (MS // 128) + j
                    for dc in range(C // 512):
                        nc.tensor.matmul(
                            o_ps[:, dc * 512:(dc + 1) * 512],
                            lhsT=pt_sb[:, j * 128:(j + 1) * 128],
                            rhs=v[:, mt, dc * 512:(dc + 1) * 512],
                            start=(mt == 0),
                            stop=(mt == mt_u - 1),
                        )

            for ntile in range(N // 128):
                partials = statp.tile([128, 4], F32, tag="partials")
                o_ps = ops.tile([128, C], F32, tag="ops")
                for g in range(M // MS):
                    s_ps = sps.tile([128, MS], F32, tag="sps")
                    for dt in range(CT):
                        nc.tensor.matmul(
                            s_ps[:],
                            lhsT=qt[:, dt, ntile * 128:(ntile + 1) * 128],
                            rhs=kT[:, dt, g * MS:(g + 1) * MS],
                            start=(dt == 0),
                            stop=(dt == CT - 1),
                        )
                    p_sb = psbp.tile([128, MS], BF16, tag="psb")
                    nc.scalar.activation(
                        p_sb[:], s_ps[:], AF.Exp,
                        scale=float(SCALE),
                        accum_out=partials[:, g:g + 1],
                    )
                    if g * MS < mv:
                        transpose_and_pv(g, p_sb, o_ps)
                denom = statp.tile([128, 1], F32, tag="denom")
                nc.vector.reduce_sum(
                    denom[:], partials[:], axis=mybir.AxisListType.X
                )
                recip = statp.tile([128, 1], F32, tag="recip")
                nc.vector.reciprocal(recip[:], denom[:])
                nc.vector.tensor_scalar_mul(
                    o_sb[:, ntile * C:(ntile + 1) * C], o_ps[:], recip[:]
                )
        # ---------------- projection with the swapaxes/reshape fold --------
        # out[2t+i, d'] = sum_c o[1024 i + c, t] pw[d', c] + bias: the lhsT
        # blocks are o's SBUF row-tiles as produced by attention.  pw is
        # dc-major like wv.  (Interleaving proj into the attention loop
        # creates tiny PE gaps that each reset the pstate ramp — net loss.)
        with (
            tc.tile_pool(name="fps", bufs=4, space="PSUM") as fps,
            tc.tile_pool(name="fsb", bufs=6) as fsbp,
        ):
            out_v = out[:].rearrange("(t two) d -> two t d", two=2)
            for i in range(2):
                for dc in range(C // 512):
                    for tt in range(CT):
                        ps = fps.tile([128, 512], F32, tag="fps")
                        for ct in range(CT):
                            nc.tensor.matmul(
                                ps[:],
                                lhsT=o_sb[:, (CT * i + ct) * C + tt * 128: (CT * i + ct) * C + (tt + 1) * 128],
                                rhs=pw[:, dc * 4096 + ct * 512: dc * 4096 + (ct + 1) * 512],
                                start=(ct == 0),
                                stop=(ct == CT - 1),
                            )
                        f_sb = fsbp.tile([128, 512], F32, tag="fsb")
                        nc.vector.tensor_add(
                            f_sb[:], ps[:], bias[:, dc * 512:(dc + 1) * 512]
                        )
                        nc.sync.dma_start(
                            out_v[i, tt * 128:(tt + 1) * 128, dc * 512:(dc + 1) * 512],
                            f_sb[:],
                        )


def _prep_w_lhs(w):
    # lhsT weights (wk, wq): dt-major [p, dt*1024 + ct*128 + dd]
    wt = w.T.reshape(CT, 128, CT, 128)          # [ct, p, dt, dd]
    return np.ascontiguousarray(
        wt.transpose(1, 2, 0, 3).reshape(128, CT * C).astype(NPBF)
    )


def _prep_w_rhs(w):
    # rhs weights (wv, pw): dc-major [p, dc*4096 + ct*512 + dd]
    wt = w.T.reshape(CT, 128, C // 512, 512)    # [ct, p, dc, dd]
    return np.ascontiguousarray(
        wt.transpose(1, 2, 0, 3).reshape(128, CT * C).astype(NPBF)
    )


def _prep_act(a, win):
    # a [rows, C] -> a.T [C, rows] grouped window-major as [p, rows/win, ct,
    # win] so each window's DMA is one contiguous per-partition run
    n = a.shape[0]
    at = a.T.reshape(CT, 128, n // win, win)    # [ct, p, w, win]
    return np.ascontiguousarray(
        at.transpose(1, 2, 0, 3).astype(NPBF)
    )


def _mask_perm(attn_mask):
    # permutation packing unmasked support rows first; tile count for packed v
    mask = np.asarray(attn_mask)
    perm = np.argsort(mask == 0, kind="stable")
    cnt = int((mask != 0).sum())
    mt_u = max(1, min(MT, -(-cnt // 128)))
    return perm, mt_u


def prep_in_maps(x, support, attn_mask, qkv_w, proj_w, proj_b):
    x = np.asarray(x, dtype=np.float32)
    support = np.asarray(support, dtype=np.float32)
    attn_mask = np.asarray(attn_mask)
    qkv_w = np.asarray(qkv_w, dtype=np.float32)
    proj_w = np.asarray(proj_w, dtype=np.float32)
    proj_b = np.asarray(proj_b, dtype=np.float32)

    perm, mt_u = _mask_perm(attn_mask)
    maskp = attn_mask[perm].astype(np.float32)

    wq = _prep_w_lhs(qkv_w[:C])
    wk = _prep_w_lhs(qkv_w[C:2 * C])
    wv = _prep_w_rhs(qkv_w[2 * C:])
    pw = _prep_w_rhs(proj_w)
    maskf = np.ascontiguousarray(
        maskp[:mt_u * 128].reshape(mt_u, 128).T
    )
    biasb = np.ascontiguousarray(np.broadcast_to(proj_b, (128, C)))

    in_maps = []
    for b in range(B):
        in_maps.append({
            "xT": _prep_act(x[b], NCHUNK),
            "sT": _prep_act(support[b][perm], MS),
            "wqT": wq,
            "wkT": wk,
            "wvT": wv,
            "pwT": pw,
            "maskf": maskf,
            "biasb": biasb,
        })
    return in_maps


def kernel(x, support, attn_mask, qkv_w, proj_w, proj_b):
    _, mt_u = _mask_perm(attn_mask)
    if ("nc", mt_u) not in _CACHE:
        _CACHE[("nc", mt_u)] = _build_program(mt_u)
        _CACHE["nc"] = _CACHE[("nc", mt_u)]
    nc = _CACHE[("nc", mt_u)]

    in_maps = prep_in_maps(x, support, attn_mask, qkv_w, proj_w, proj_b)
    res = run_bass_kernel_spmd(nc, in_maps, core_ids=list(range(B)))
    return np.stack([res.results[b]["out"] for b in range(B)], axis=0)
